# revision 12
# baseline (speedup 1.0000x reference)
"""Binary conv (XNOR-style) 3x3 + sync-BN on 8 Trainium2 NeuronCores.

Problem: x[32,256,56,56], w[256,256,3,3] -> sign(x) conv sign(w), pad 1,
then BatchNorm (training mode, global batch stats) with gamma/beta.

Sharding: data-parallel over batch (4 images per core, 8 cores). BN batch
stats are made global with a tiny (2 KB) AllReduce of per-channel
sum / sum-of-squares (sync-BN), so the result matches single-device math.

Per-core kernel:
  - binarize w (sign, ACT engine) into fp8 lhsT layout [ci, tap, co]
  - per image: DMA x (split per ci-tile), binarize into a zero-padded
    [60*58] fp8 buffer
  - conv as shifted-window implicit GEMM: output pixels live at padded
    flat positions p in [58, 3306); for tap (kh,kw) the input is the
    padded buffer shifted by 58*kh + kw - 59. DoubleRow fp8 matmuls
    contract all 256 input channels at once (two 128-partition tiles
    interleaved); taps outer / chunks (464 cols = 8 padded rows) inner
    so one LDWEIGHTS serves a half-group of chunks, PSUM double-buffered
    in halves. Conv values are small integers: exact in fp8/fp32.
  - PSUM -> SBUF fp16 y (exact: |y| <= 2304, realistically < 2048)
  - per-channel sum (DVE reduce) + sum(y^2)/64 (ACT Square w/ fp32
    accumulator), folded on ACT, AllReduce over 8 cores, rsqrt via
    reciprocal+sqrt+Newton, then y*scale+bias -> fp32 out.
"""

import os
import numpy as np

import concourse.bass as bass
import concourse.mybir as mybir
import concourse.tile as tile
from concourse import bacc
from concourse import bass_utils

F32 = mybir.dt.float32
F16 = mybir.dt.float16
BF16 = mybir.dt.bfloat16
F8 = mybir.dt.float8e4

N_CORES = 8
NL = 4            # images per core
CI = 256          # input channels
CO = 256          # output channels
H = W = 56
HP = 58           # padded row length
PIX = H * W       # 3136
ZROWS = 60        # padded buffer rows (58 used + slack so 3480 = 60*58)
ZLEN = ZROWS * HP # 3480
ZPAD = 3488       # fp8 per-ci-tile stride; %16 == 0 for DoubleRow APs
CHUNK = 464       # 8 padded rows per matmul free-dim chunk
NCHUNK = 7        # 7*464 = 3248 = 56*58 computed positions [58, 3306)
YLEN = NCHUNK * CHUNK  # 3248
VCHUNK = 448      # valid cols per chunk (8 rows x 56)
VLEN = NCHUNK * VCHUNK  # 3136
NTOT_PIX = 32 * PIX    # BN normalizer (full batch)
BN_EPS = 1e-5
SSQ_SCALE = 1.0 / 64.0  # keep y^2/64 in fp16 range in the junk output


def _build(timing_proxy: bool = False):
    nc = bacc.Bacc("TRN2", target_bir_lowering=False, debug=False,
                   num_devices=N_CORES)

    xs = nc.dram_tensor("xs", [NL, CI, H, W], F32, kind="ExternalInput").ap()
    wt = nc.dram_tensor("wt", [CI, 9, CO], F32, kind="ExternalInput").ap()
    gamma = nc.dram_tensor("gamma", [CO], F32, kind="ExternalInput").ap()
    beta = nc.dram_tensor("beta", [CO], F32, kind="ExternalInput").ap()
    o = nc.dram_tensor("o", [NL, CO, H, W], F32, kind="ExternalOutput").ap()

    with tile.TileContext(nc) as tc:
        with (
            tc.tile_pool(name="wpool", bufs=1) as wpool,
            tc.tile_pool(name="xpool", bufs=4) as xpool,
            tc.tile_pool(name="zpool", bufs=2) as zpool,
            tc.tile_pool(name="ypool", bufs=1) as ypool,
            tc.tile_pool(name="spool", bufs=1) as spool,
            tc.tile_pool(name="jpool", bufs=2) as jpool,
            tc.tile_pool(name="opool", bufs=3) as opool,
            tc.tile_pool(name="psum", bufs=8, space="PSUM") as psum_pool,
            tc.tile_pool(name="dram", bufs=1, space="DRAM") as dram,
        ):
            # ---- weights: DMA fp32 [ci, tap, co] -> sign -> fp8 ----
            w_f32 = wpool.tile([128, 2, 9, CO], F32, tag="wf32")
            w_bin = wpool.tile([128, 2, 9, CO], F8, tag="wbin")
            wt_r = wt.rearrange("(ct p) t co -> p ct t co", p=128)
            for t0, t1 in ((0, 5), (5, 9)):
                nc.sync.dma_start(w_f32[:, :, t0:t1, :], wt_r[:, :, t0:t1, :])
                nc.scalar.sign(w_bin[:, :, t0:t1, :], w_f32[:, :, t0:t1, :])

            # preload the sqrt ACT table set off the critical path
            sqwarm = spool.tile([128, 1], F32, tag="sqwarm")
            nc.vector.memset(sqwarm[:], 1.0)
            nc.scalar.sqrt(sqwarm[:], sqwarm[:])

            # gamma/beta per-partition: channel c = t*128 + p
            gb_g = spool.tile([128, 2], F32, tag="gb_g")
            gb_b = spool.tile([128, 2], F32, tag="gb_b")
            nc.sync.dma_start(gb_g[:], gamma.rearrange("(t p) -> p t", p=128))
            nc.sync.dma_start(gb_b[:], beta.rearrange("(t p) -> p t", p=128))

            # ---- persistent per-image state ----
            ys = ypool.tile([128, 2, NL, VLEN], F16, tag="ys")
            sums = spool.tile([128, 2, NL, NCHUNK], F32, tag="sums")
            ssqs = spool.tile([128, 2, NL, NCHUNK], F32, tag="ssqs")

            for n in range(NL):
                z = zpool.tile([128, 2, ZPAD], F8, tag="z")
                nc.gpsimd.memset(z[:], 0.0)
                z58 = z[:, :, 0:ZLEN].rearrange("p c (r q) -> p c r q", q=HP)
                row_splits = ((0, 28), (28, 56)) if n == 0 else ((0, 56),)
                for ct in range(2):
                    for (r0, r1) in row_splits:
                        xst = xpool.tile([128, H, W], F32, tag="xst",
                                         name=f"xst_{n}_{ct}_{r0}")
                        nc.sync.dma_start(
                            xst[:, 0:r1 - r0, :],
                            xs[n, ct * 128:(ct + 1) * 128, r0:r1])
                        # padded interior: flat idx 58*(h+1)+(w+2)
                        nc.scalar.sign(z58[:, ct, 1 + r0:1 + r1, 2:58],
                                       xst[:, 0:r1 - r0, :])

                # conv: DoubleRow fp8 contracts both ci tiles (K=256) at
                # once; taps outer / chunks inner so one LDWEIGHTS serves
                # a half-group of chunks (PSUM double-buffers in halves).
                for cot in range(2):
                    cos = slice(cot * 128, (cot + 1) * 128)
                    for half, cs in ((0, range(0, 4)), (1, range(4, 7))):
                        accs = {
                            c: psum_pool.tile([128, CHUNK], F32, tag="acc",
                                              name=f"acc_{n}_{cot}_{c}")
                            for c in cs
                        }
                        for t in range(9):
                            kh, kw = t // 3, t % 3
                            for c in cs:
                                off = CHUNK * c + HP * kh + kw
                                nc.tensor.matmul(
                                    accs[c][:],
                                    w_bin[:, :, t, cos],
                                    z[:, :, off:off + CHUNK],
                                    start=(t == 0), stop=(t == 8),
                                    perf_mode=mybir.MatmulPerfMode.DoubleRow,
                                )
                        for c in cs:
                            dst = ys[:, cot, n,
                                     VCHUNK * c:VCHUNK * (c + 1)]
                            dst3 = dst.rearrange("p (r q) -> p r q", q=56)
                            src3 = accs[c].rearrange(
                                "p (r q) -> p r q", q=HP)[:, :, 1:57]
                            nc.vector.tensor_scalar(
                                dst3, src3, 1.0, 0.0,
                                op0=mybir.AluOpType.mult,
                                op1=mybir.AluOpType.add,
                                accum_out=sums[:, cot, n, c:c + 1])
                            junk = jpool.tile([128, VCHUNK], F16, tag="junk",
                                              name=f"junk_{n}_{cot}_{c}")
                            # sum(y^2)/64 via ACT Square(y/8), fp32 accum
                            # (tensor_tensor_reduce faults this HW)
                            nc.scalar.activation(
                                junk[:], dst,
                                mybir.ActivationFunctionType.Square,
                                scale=0.125,
                                accum_out=ssqs[:, cot, n, c:c + 1])


            # ---- sync-BN: AllReduce per-channel [sum, sum(y^2)] ----
            # Cross-image fold stays on ACT (accum_out results must not be
            # read by another engine directly on this HW; ACT reading its
            # own accum tiles is serialized and safe). Copy barrier before
            # the DMA for the same reason.
            cc_stage = spool.tile([128, 4], F32, tag="cc_stage")
            jk2 = spool.tile([128, NL * NCHUNK], F32, tag="jk2")
            sums_b = spool.tile([128, 2, NL, NCHUNK], F32, tag="sums_b")
            nc.vector.tensor_copy(sums_b[:], sums[:])
            for cot in range(2):
                nc.scalar.activation(
                    jk2[:], sums_b[:, cot, :, :],
                    mybir.ActivationFunctionType.Copy,
                    accum_out=cc_stage[:, cot:cot + 1])
                nc.scalar.activation(
                    jk2[:], ssqs[:, cot, :, :],
                    mybir.ActivationFunctionType.Copy,
                    scale=1.0 / SSQ_SCALE,
                    accum_out=cc_stage[:, 2 + cot:3 + cot])
            cc_stage2 = spool.tile([128, 4], F32, tag="cc_stage2")
            nc.scalar.copy(cc_stage2[:], cc_stage[:])
            cc_in = dram.tile([128, 4], F32, tag="cc_in")
            cc_out = dram.tile([128, 4], F32, tag="cc_out")
            nc.sync.dma_start(cc_in[:], cc_stage2[:])
            if timing_proxy:
                nc.sync.dma_start(cc_out[:], cc_in[:])
            else:
                nc.gpsimd.collective_compute(
                    "AllReduce",
                    mybir.AluOpType.add,
                    replica_groups=[list(range(N_CORES))],
                    ins=[cc_in.opt()],
                    outs=[cc_out.opt()],
                )
            gstat = spool.tile([128, 4], F32, tag="gstat")
            nc.sync.dma_start(gstat[:], cc_out[:])

            # ---- finalize: mean/var -> scale/bias (tiny [128,2] ops) ----
            mean = spool.tile([128, 2], F32, tag="mean")
            ey2e = spool.tile([128, 2], F32, tag="ey2e")
            var = spool.tile([128, 2], F32, tag="var")
            r0 = spool.tile([128, 2], F32, tag="r0")
            t1 = spool.tile([128, 2], F32, tag="t1")
            sc = spool.tile([128, 2], F32, tag="sc")
            bs = spool.tile([128, 2], F32, tag="bs")
            nc.vector.tensor_scalar_mul(mean[:], gstat[:, 0:2], 1.0 / NTOT_PIX)
            nc.vector.tensor_scalar(ey2e[:], gstat[:, 2:4],
                                    1.0 / NTOT_PIX, BN_EPS,
                                    op0=mybir.AluOpType.mult,
                                    op1=mybir.AluOpType.add)
            nc.vector.tensor_tensor(var[:], mean[:], mean[:],
                                    op=mybir.AluOpType.mult)
            nc.vector.tensor_tensor(var[:], ey2e[:], var[:],
                                    op=mybir.AluOpType.subtract)
            # inv = rsqrt(var+eps): sqrt(1/v) then one Newton step
            nc.vector.reciprocal(r0[:], var[:])
            nc.scalar.sqrt(r0[:], r0[:])
            nc.vector.tensor_tensor(t1[:], r0[:], r0[:],
                                    op=mybir.AluOpType.mult)
            nc.vector.tensor_tensor(t1[:], t1[:], var[:],
                                    op=mybir.AluOpType.mult)
            nc.vector.tensor_scalar(t1[:], t1[:], -0.5, 1.5,
                                    op0=mybir.AluOpType.mult,
                                    op1=mybir.AluOpType.add)
            nc.vector.tensor_tensor(r0[:], r0[:], t1[:],
                                    op=mybir.AluOpType.mult)
            nc.vector.tensor_tensor(sc[:], gb_g[:], r0[:],
                                    op=mybir.AluOpType.mult)
            nc.vector.tensor_tensor(t1[:], mean[:], sc[:],
                                    op=mybir.AluOpType.mult)
            nc.vector.tensor_tensor(bs[:], gb_b[:], t1[:],
                                    op=mybir.AluOpType.subtract)

            # ---- normalize + store ----
            for n in range(NL):
                for cot in range(2):
                    ost = opool.tile([128, H, W], F32, tag="ost",
                                     name=f"ost_{n}_{cot}")
                    yv = ys[:, cot, n, :].rearrange("p (h w) -> p h w", w=W)
                    if (2 * n + cot) % 8 not in (1, 4, 7):
                        nc.vector.tensor_scalar(
                            ost[:], yv, sc[:, cot:cot + 1], bs[:, cot:cot + 1],
                            op0=mybir.AluOpType.mult, op1=mybir.AluOpType.add)
                    else:
                        nc.scalar.activation(
                            ost[:], yv, mybir.ActivationFunctionType.Identity,
                            bias=bs[:, cot:cot + 1], scale=sc[:, cot:cot + 1])
                    nc.sync.dma_start(o[n, cot * 128:(cot + 1) * 128], ost[:])

    nc.compile()
    return nc


_CACHE: dict = {}


def _get_nc():
    key = "proxy" if os.environ.get("BK_TIMING_PROXY") == "1" else "real"
    if key not in _CACHE:
        _CACHE[key] = _build(timing_proxy=(key == "proxy"))
    return _CACHE[key]


def kernel(x, w, gamma, beta):
    x = np.ascontiguousarray(np.asarray(x, dtype=np.float32))
    w = np.asarray(w, dtype=np.float32)
    gamma = np.ascontiguousarray(np.asarray(gamma, dtype=np.float32))
    beta = np.ascontiguousarray(np.asarray(beta, dtype=np.float32))
    # host-side layout only (no math): [co,ci,kh,kw] -> [ci, kh*kw, co]
    w_t = np.ascontiguousarray(w.transpose(1, 2, 3, 0).reshape(CI, 9, CO))

    nc = _get_nc()
    in_maps = [
        {"xs": x[NL * c:NL * (c + 1)], "wt": w_t, "gamma": gamma, "beta": beta}
        for c in range(N_CORES)
    ]
    res = bass_utils.run_bass_kernel_spmd(
        nc, in_maps, core_ids=list(range(N_CORES)))
    return np.concatenate([res.results[c]["o"] for c in range(N_CORES)], axis=0)


# revision 14
# speedup vs baseline: 1.0656x; 1.0656x over previous
"""Binary conv (XNOR-style) 3x3 + sync-BN on 8 Trainium2 NeuronCores.

Problem: x[32,256,56,56], w[256,256,3,3] -> sign(x) conv sign(w), pad 1,
then BatchNorm (training mode, global batch stats) with gamma/beta.

Sharding: data-parallel over batch (4 images per core, 8 cores). BN batch
stats are made global with a tiny (2 KB) AllReduce of per-channel
sum / sum-of-squares (sync-BN), so the result matches single-device math.

Per-core kernel:
  - binarize w (sign, ACT engine) into fp8 lhsT layout [ci, tap, co]
  - per image: DMA x (split per ci-tile), binarize into a zero-padded
    [60*58] fp8 buffer
  - conv as shifted-window implicit GEMM: output pixels live at padded
    flat positions p in [58, 3306); for tap (kh,kw) the input is the
    padded buffer shifted by 58*kh + kw - 59. DoubleRow fp8 matmuls
    contract all 256 input channels at once (two 128-partition tiles
    interleaved); taps outer / chunks (464 cols = 8 padded rows) inner
    so one LDWEIGHTS serves a half-group of chunks, PSUM double-buffered
    in halves. Conv values are small integers: exact in fp8/fp32.
  - PSUM -> SBUF fp16 y (exact: |y| <= 2304, realistically < 2048)
  - per-channel sum (DVE reduce) + sum(y^2)/64 (ACT Square w/ fp32
    accumulator), folded on ACT, AllReduce over 8 cores, rsqrt via
    reciprocal+sqrt+Newton, then y*scale+bias -> fp32 out.
"""

import os
import numpy as np

import concourse.bass as bass
import concourse.mybir as mybir
import concourse.tile as tile
from concourse import bacc
from concourse import bass_utils

F32 = mybir.dt.float32
F16 = mybir.dt.float16
BF16 = mybir.dt.bfloat16
F8 = mybir.dt.float8e4

N_CORES = 8
NL = 4            # images per core
CI = 256          # input channels
CO = 256          # output channels
H = W = 56
HP = 58           # padded row length
PIX = H * W       # 3136
ZROWS = 60        # padded buffer rows (58 used + slack so 3480 = 60*58)
ZLEN = ZROWS * HP # 3480
ZPAD = 3488       # fp8 per-ci-tile stride; %16 == 0 for DoubleRow APs
CHUNK = 464       # 8 padded rows per matmul free-dim chunk
NCHUNK = 7        # 7*464 = 3248 = 56*58 computed positions [58, 3306)
YLEN = NCHUNK * CHUNK  # 3248
VCHUNK = 448      # valid cols per chunk (8 rows x 56)
VLEN = NCHUNK * VCHUNK  # 3136
NTOT_PIX = 32 * PIX    # BN normalizer (full batch)
BN_EPS = 1e-5
SSQ_SCALE = 1.0 / 64.0  # keep y^2/64 in fp16 range in the junk output


def _build(timing_proxy: bool = False):
    nc = bacc.Bacc("TRN2", target_bir_lowering=False, debug=False,
                   num_devices=N_CORES)

    xs = nc.dram_tensor("xs", [NL, CI, H, W], F32, kind="ExternalInput").ap()
    wt = nc.dram_tensor("wt", [CI, 9, CO], F32, kind="ExternalInput").ap()
    gamma = nc.dram_tensor("gamma", [CO], F32, kind="ExternalInput").ap()
    beta = nc.dram_tensor("beta", [CO], F32, kind="ExternalInput").ap()
    o = nc.dram_tensor("o", [NL, CO, H, W], F32, kind="ExternalOutput").ap()

    with tile.TileContext(nc) as tc:
        with (
            tc.tile_pool(name="wpool", bufs=1) as wpool,
            tc.tile_pool(name="xpool", bufs=4) as xpool,
            tc.tile_pool(name="zpool", bufs=2) as zpool,
            tc.tile_pool(name="ypool", bufs=1) as ypool,
            tc.tile_pool(name="spool", bufs=1) as spool,
            tc.tile_pool(name="jpool", bufs=2) as jpool,
            tc.tile_pool(name="opool", bufs=3) as opool,
            tc.tile_pool(name="psum", bufs=8, space="PSUM") as psum_pool,
            tc.tile_pool(name="dram", bufs=1, space="DRAM") as dram,
        ):
            # ---- weights: DMA fp32 [ci, tap, co] -> sign -> fp8 ----
            w_f32 = wpool.tile([128, 2, 9, CO], F32, tag="wf32")
            w_bin = wpool.tile([128, 2, 9, CO], F8, tag="wbin")
            nc.sync.dma_start(
                w_f32[:], wt.rearrange("(ct p) t co -> p ct t co", p=128))
            nc.scalar.sign(w_bin[:], w_f32[:])

            # preload the sqrt ACT table set off the critical path
            sqwarm = spool.tile([128, 1], F32, tag="sqwarm")
            nc.vector.memset(sqwarm[:], 1.0)
            nc.scalar.sqrt(sqwarm[:], sqwarm[:])

            # gamma/beta per-partition: channel c = t*128 + p
            gb_g = spool.tile([128, 2], F32, tag="gb_g")
            gb_b = spool.tile([128, 2], F32, tag="gb_b")
            nc.sync.dma_start(gb_g[:], gamma.rearrange("(t p) -> p t", p=128))
            nc.sync.dma_start(gb_b[:], beta.rearrange("(t p) -> p t", p=128))

            # ---- persistent per-image state ----
            ys = ypool.tile([128, 2, NL, VLEN], F16, tag="ys")
            sums = spool.tile([128, 2, NL, NCHUNK], F32, tag="sums")
            ssqs = spool.tile([128, 2, NL], F32, tag="ssqs")

            for n in range(NL):
                z = zpool.tile([128, 2, ZPAD], F8, tag="z")
                nc.gpsimd.memset(z[:], 0.0)
                z58 = z[:, :, 0:ZLEN].rearrange("p c (r q) -> p c r q", q=HP)
                row_splits = ((0, 28), (28, 56)) if n == 0 else ((0, 56),)
                for ct in range(2):
                    for (r0, r1) in row_splits:
                        xst = xpool.tile([128, H, W], F32, tag="xst",
                                         name=f"xst_{n}_{ct}_{r0}")
                        nc.sync.dma_start(
                            xst[:, 0:r1 - r0, :],
                            xs[n, ct * 128:(ct + 1) * 128, r0:r1])
                        # padded interior: flat idx 58*(h+1)+(w+2)
                        nc.scalar.sign(z58[:, ct, 1 + r0:1 + r1, 2:58],
                                       xst[:, 0:r1 - r0, :])

                # conv: DoubleRow fp8 contracts both ci tiles (K=256) at
                # once; taps outer / chunks inner so one LDWEIGHTS serves
                # a half-group of chunks (PSUM double-buffers in halves).
                for cot in range(2):
                    cos = slice(cot * 128, (cot + 1) * 128)
                    for half, cs in ((0, range(0, 4)), (1, range(4, 7))):
                        accs = {
                            c: psum_pool.tile([128, CHUNK], F32, tag="acc",
                                              name=f"acc_{n}_{cot}_{c}")
                            for c in cs
                        }
                        for t in range(9):
                            kh, kw = t // 3, t % 3
                            for c in cs:
                                off = CHUNK * c + HP * kh + kw
                                nc.tensor.matmul(
                                    accs[c][:],
                                    w_bin[:, :, t, cos],
                                    z[:, :, off:off + CHUNK],
                                    start=(t == 0), stop=(t == 8),
                                    perf_mode=mybir.MatmulPerfMode.DoubleRow,
                                )
                        for c in cs:
                            dst = ys[:, cot, n,
                                     VCHUNK * c:VCHUNK * (c + 1)]
                            dst3 = dst.rearrange("p (r q) -> p r q", q=56)
                            src3 = accs[c].rearrange(
                                "p (r q) -> p r q", q=HP)[:, :, 1:57]
                            nc.vector.tensor_scalar(
                                dst3, src3, 1.0, 0.0,
                                op0=mybir.AluOpType.mult,
                                op1=mybir.AluOpType.add,
                                accum_out=sums[:, cot, n, c:c + 1])

                # ---- per-image sum(y^2) from contiguous ys ----
                for cot in range(2):
                    junk = jpool.tile([128, VLEN], F16, tag="junk",
                                      name=f"junk_{n}_{cot}")
                    # Square(y/8) w/ fp32 accum = sum(y^2)/64
                    # (tensor_tensor_reduce faults this HW)
                    nc.scalar.activation(
                        junk[:], ys[:, cot, n, :],
                        mybir.ActivationFunctionType.Square,
                        scale=0.125,
                        accum_out=ssqs[:, cot, n:n + 1])

            # ---- sync-BN: AllReduce per-channel [sum, sum(y^2)] ----
            # Cross-image fold stays on ACT (accum_out results must not be
            # read by another engine directly on this HW; ACT reading its
            # own accum tiles is serialized and safe). Copy barrier before
            # the DMA for the same reason.
            cc_stage = spool.tile([128, 4], F32, tag="cc_stage")
            jk2 = spool.tile([128, NL * NCHUNK], F32, tag="jk2")
            sums_b = spool.tile([128, 2, NL, NCHUNK], F32, tag="sums_b")
            nc.vector.tensor_copy(sums_b[:], sums[:])
            for cot in range(2):
                nc.scalar.activation(
                    jk2[:], sums_b[:, cot, :, :],
                    mybir.ActivationFunctionType.Copy,
                    accum_out=cc_stage[:, cot:cot + 1])
                nc.scalar.activation(
                    jk2[:, 0:NL], ssqs[:, cot, :],
                    mybir.ActivationFunctionType.Copy,
                    scale=1.0 / SSQ_SCALE,
                    accum_out=cc_stage[:, 2 + cot:3 + cot])
            cc_stage2 = spool.tile([128, 4], F32, tag="cc_stage2")
            nc.scalar.copy(cc_stage2[:], cc_stage[:])
            cc_in = dram.tile([128, 4], F32, tag="cc_in")
            cc_out = dram.tile([128, 4], F32, tag="cc_out")
            nc.sync.dma_start(cc_in[:], cc_stage2[:])
            if timing_proxy:
                nc.sync.dma_start(cc_out[:], cc_in[:])
            else:
                nc.gpsimd.collective_compute(
                    "AllReduce",
                    mybir.AluOpType.add,
                    replica_groups=[list(range(N_CORES))],
                    ins=[cc_in.opt()],
                    outs=[cc_out.opt()],
                )
            gstat = spool.tile([128, 4], F32, tag="gstat")
            nc.sync.dma_start(gstat[:], cc_out[:])

            # ---- finalize: mean/var -> scale/bias (tiny [128,2] ops) ----
            mean = spool.tile([128, 2], F32, tag="mean")
            ey2e = spool.tile([128, 2], F32, tag="ey2e")
            var = spool.tile([128, 2], F32, tag="var")
            r0 = spool.tile([128, 2], F32, tag="r0")
            t1 = spool.tile([128, 2], F32, tag="t1")
            sc = spool.tile([128, 2], F32, tag="sc")
            bs = spool.tile([128, 2], F32, tag="bs")
            nc.vector.tensor_scalar_mul(mean[:], gstat[:, 0:2], 1.0 / NTOT_PIX)
            nc.vector.tensor_scalar(ey2e[:], gstat[:, 2:4],
                                    1.0 / NTOT_PIX, BN_EPS,
                                    op0=mybir.AluOpType.mult,
                                    op1=mybir.AluOpType.add)
            nc.vector.tensor_tensor(var[:], mean[:], mean[:],
                                    op=mybir.AluOpType.mult)
            nc.vector.tensor_tensor(var[:], ey2e[:], var[:],
                                    op=mybir.AluOpType.subtract)
            # inv = rsqrt(var+eps): sqrt(1/v) then one Newton step
            nc.vector.reciprocal(r0[:], var[:])
            nc.scalar.sqrt(r0[:], r0[:])
            nc.vector.tensor_tensor(t1[:], r0[:], r0[:],
                                    op=mybir.AluOpType.mult)
            nc.vector.tensor_tensor(t1[:], t1[:], var[:],
                                    op=mybir.AluOpType.mult)
            nc.vector.tensor_scalar(t1[:], t1[:], -0.5, 1.5,
                                    op0=mybir.AluOpType.mult,
                                    op1=mybir.AluOpType.add)
            nc.vector.tensor_tensor(r0[:], r0[:], t1[:],
                                    op=mybir.AluOpType.mult)
            nc.vector.tensor_tensor(sc[:], gb_g[:], r0[:],
                                    op=mybir.AluOpType.mult)
            nc.vector.tensor_tensor(t1[:], mean[:], sc[:],
                                    op=mybir.AluOpType.mult)
            nc.vector.tensor_tensor(bs[:], gb_b[:], t1[:],
                                    op=mybir.AluOpType.subtract)

            # ---- normalize + store ----
            for n in range(NL):
                for cot in range(2):
                    ost = opool.tile([128, H, W], F32, tag="ost",
                                     name=f"ost_{n}_{cot}")
                    yv = ys[:, cot, n, :].rearrange("p (h w) -> p h w", w=W)
                    if (2 * n + cot) % 8 not in (1, 4, 7):
                        nc.vector.tensor_scalar(
                            ost[:], yv, sc[:, cot:cot + 1], bs[:, cot:cot + 1],
                            op0=mybir.AluOpType.mult, op1=mybir.AluOpType.add)
                    else:
                        nc.scalar.activation(
                            ost[:], yv, mybir.ActivationFunctionType.Identity,
                            bias=bs[:, cot:cot + 1], scale=sc[:, cot:cot + 1])
                    nc.sync.dma_start(o[n, cot * 128:(cot + 1) * 128], ost[:])

    nc.compile()
    return nc


_CACHE: dict = {}


def _get_nc():
    key = "proxy" if os.environ.get("BK_TIMING_PROXY") == "1" else "real"
    if key not in _CACHE:
        _CACHE[key] = _build(timing_proxy=(key == "proxy"))
    return _CACHE[key]


def kernel(x, w, gamma, beta):
    x = np.ascontiguousarray(np.asarray(x, dtype=np.float32))
    w = np.asarray(w, dtype=np.float32)
    gamma = np.ascontiguousarray(np.asarray(gamma, dtype=np.float32))
    beta = np.ascontiguousarray(np.asarray(beta, dtype=np.float32))
    # host-side layout only (no math): [co,ci,kh,kw] -> [ci, kh*kw, co]
    w_t = np.ascontiguousarray(w.transpose(1, 2, 3, 0).reshape(CI, 9, CO))

    nc = _get_nc()
    in_maps = [
        {"xs": x[NL * c:NL * (c + 1)], "wt": w_t, "gamma": gamma, "beta": beta}
        for c in range(N_CORES)
    ]
    res = bass_utils.run_bass_kernel_spmd(
        nc, in_maps, core_ids=list(range(N_CORES)))
    return np.concatenate([res.results[c]["o"] for c in range(N_CORES)], axis=0)
